# revision 7
# baseline (speedup 1.0000x reference)
"""Multi-head attention Bass kernel for Trainium2 (8 NeuronCores).

Problem: B=2, N=4096, E=768, H=12 heads of dim 64 (nn_MultiHeadAttention).
Sharding: 2 batches x 4 head-groups (3 heads each) = 8 cores. Each core:
  - QKV projection for its 3 heads (x pre-transposed on host to [E, N])
  - flash-style attention with transposed scores P[kv, q] (no max subtraction:
    scores are tightly bounded ~N(0, 0.3^2) for this problem's scale)
  - softmax denominators via a ones-column appended to V in the P@V matmul
  - output projection against its 192 w_proj rows -> partial [N, 768]
Host: sums the 4 partials per batch and adds the (bias-folded) b_proj.

Bias handling (exact algebra, no approximation):
  - K bias drops out of softmax (adds a per-query constant to scores).
  - V bias commutes through P@V normalization; bv @ w_proj.T folds into b_proj.
  - Q bias is applied on device (per-partition bias in the QKV->SBUF copy).
"""

import sys

sys.path.insert(0, "/opt/trn_rl_repo")

import numpy as np

import concourse.bass as bass  # noqa: E402
import concourse.mybir as mybir  # noqa: E402
import concourse.tile as tile  # noqa: E402
from concourse import bacc  # noqa: E402
from concourse.bass_utils import run_bass_kernel_spmd  # noqa: E402

F32 = mybir.dt.float32
F32R = mybir.dt.float32r


def _r(ap):
    """Bitcast an fp32 AP to float32r for full-rate PE matmuls."""
    return ap.bitcast(F32R)
AF = mybir.ActivationFunctionType

B, N, E = 2, 4096, 768
H, HD = 12, 64
NH = 3          # heads per core
M_GROUPS = 4    # head groups (tensor parallel)
GD = NH * HD    # 192 y-dims per core
GDP = 256       # V matmul moving dim padded to 256 (f32r full-rate needs >=256)
QKDIM = 2 * NH * HD  # 384 qk output dims per core


def build_nc(n_tokens=N, num_devices=8):
    """Build the per-core Bass module (SPMD: same program, different data)."""
    n = n_tokens
    NQG = n // 512          # q groups of 512
    NKV = n // 128          # kv blocks of 128
    KE = E // 128           # contraction tiles over E

    nc = bacc.Bacc("TRN2", target_bir_lowering=False, debug=False,
                   num_devices=num_devices)

    xT = nc.dram_tensor("xT", [E, n], F32R, kind="ExternalInput")
    wqkT = nc.dram_tensor("wqkT", [E, QKDIM], F32R, kind="ExternalInput")
    wvT = nc.dram_tensor("wvT", [E, GDP], F32R, kind="ExternalInput")
    bq = nc.dram_tensor("bq", [2, 128], F32, kind="ExternalInput")
    wpT = nc.dram_tensor("wpT", [HD, NH, E], F32R, kind="ExternalInput")
    out = nc.dram_tensor("out", [n, E], F32, kind="ExternalOutput")

    with tile.TileContext(nc) as tc:
        with (
            tc.tile_pool(name="perm", bufs=1) as perm,
            tc.tile_pool(name="wpool", bufs=1) as wpool,
        ):
            # Persistent SBUF tensors
            # qk_sb[:, j, 0:n] = Q.T area, [:, j, n:2n] = K.T area.
            # j=0: head0 on partitions 0:64, head1 on 64:128; j=1: head2 on 0:64.
            qk_sb = perm.tile([128, 2, 2 * n], F32R)
            # V (+ ones col per head) in [kv, d] layout: per kv-block of 128
            # tokens, 3 heads x (64 dims + ones col).
            v_sb = perm.tile([128, NKV, NH * (HD + 1)], F32R)

            wqkT_sb = wpool.tile([128, KE, QKDIM], F32R)
            wvT_sb = wpool.tile([128, KE, GDP], F32R)
            wpT_sb = wpool.tile([64, NH, E], F32R)
            bq_sb = wpool.tile([128, 2], F32)

            nc.sync.dma_start(wqkT_sb[:], wqkT.rearrange("(a p) c -> p a c", p=128))
            nc.sync.dma_start(wvT_sb[:], wvT.rearrange("(a p) c -> p a c", p=128))
            nc.sync.dma_start(wpT_sb[:], wpT[:])
            nc.sync.dma_start(bq_sb[:], bq.rearrange("a p -> p a"))

            # ones columns for the softmax-denominator trick
            ones_view = v_sb.rearrange("p a (h c) -> p a h c", c=HD + 1)[:, :, :, HD:]
            nc.vector.memset(ones_view.bitcast(F32), 1.0)

            # ---- Stage A: QKV projections ----
            with (
                tc.tile_pool(name="apsum", bufs=1, space="PSUM") as apsum,
                tc.tile_pool(name="xpool", bufs=3) as xpool,
            ):
                for ng in range(NQG):
                    psq = [apsum.tile([128, 512], F32, tag="qkt", bufs=3,
                                      name=f"psq{ng}_{m}") for m in range(3)]
                    psv = [apsum.tile([128, GDP], F32, tag="v", bufs=4,
                                      name=f"psv{ng}_{j}") for j in range(4)]
                    for k in range(KE):
                        xt = xpool.tile([128, 512], F32R, tag="xt", name=f"xt{ng}_{k}")
                        nc.sync.dma_start(xt[:], xT[k * 128:(k + 1) * 128,
                                                    ng * 512:(ng + 1) * 512])
                        st, sp = (k == 0), (k == KE - 1)
                        for m in range(3):
                            nc.tensor.matmul(psq[m][:],
                                             wqkT_sb[:, k, m * 128:(m + 1) * 128],
                                             xt[:], start=st, stop=sp)
                        for j in range(4):
                            nc.tensor.matmul(psv[j][:],
                                             xt[:, j * 128:(j + 1) * 128],
                                             wvT_sb[:, k, :], start=st, stop=sp)
                    qs = slice(ng * 512, (ng + 1) * 512)
                    ks = slice(n + ng * 512, n + (ng + 1) * 512)
                    # Q head0/1 (+bias) ; K head0/1 ; Q head2 (+bias) ; K head2
                    nc.scalar.add(qk_sb[:, 0, qs], psq[0][:], bq_sb[:, 0:1])
                    nc.vector.tensor_copy(qk_sb[:, 0, ks], psq[1][:])
                    nc.scalar.add(qk_sb[0:64, 1, qs], psq[2][0:64, :],
                                  bq_sb[0:64, 1:2])
                    # K head2 must live on partitions 0:64 (same as its Q).
                    # DMA can't read PSUM, so stage in SBUF then do a
                    # partition-shifting SBUF->SBUF DMA.
                    k2st = xpool.tile([128, 512], F32R, tag="k2st",
                                      name=f"k2st{ng}")
                    nc.vector.tensor_copy(k2st[64:128, :], psq[2][64:128, :])
                    nc.sync.dma_start(qk_sb[0:64, 1, ks], k2st[64:128, :])
                    for j in range(4):
                        dst = v_sb[:, ng * 4 + j:ng * 4 + j + 1, :].rearrange(
                            "p a (h c) -> p (a h) c", c=HD + 1)[:, :, 0:HD]
                        src = psv[j][:, 0:GD].rearrange("p (h c) -> p h c", c=HD)
                        nc.vector.tensor_copy(dst, src)

            # ---- Stage B + C: attention + projection, per q-group ----
            with (
                tc.tile_pool(name="bpsum", bufs=1, space="PSUM") as bpsum,
                tc.tile_pool(name="spool", bufs=3) as spool,
            ):
                for qg in range(NQG):
                    qsl = slice(qg * 512, (qg + 1) * 512)
                    pv = {}
                    yn = {}

                    def head_attn(h, jblk, pbase, paired_with=None):
                        """Emit scores+exp+PV for head h (and optionally a
                        row-group-disjoint partner head for PE concurrency)."""
                        heads = [(h, jblk, pbase)]
                        if paired_with is not None:
                            heads.append(paired_with)
                        for hh, _, _ in heads:
                            pv[hh] = bpsum.tile([HD + 1, 512], F32, tag="pv",
                                                bufs=2, name=f"pv{qg}_{hh}")
                        for kp in range(NKV // 2):
                            scs = {}
                            for hh, jb, pb in heads:
                                scs[hh] = bpsum.tile([128, 2, 512], F32,
                                                     tag="sc", bufs=2,
                                                     name=f"sc{qg}_{hh}_{kp}")
                            for j in range(2):
                                kv = 2 * kp + j
                                for hh, jb, pb in heads:
                                    lhs = qk_sb[pb:pb + 64, jb,
                                                n + kv * 128:n + (kv + 1) * 128]
                                    rhs = qk_sb[pb:pb + 64, jb, qsl]
                                    nc.tensor.matmul(scs[hh][:, j, :], lhs,
                                                     rhs, start=True, stop=True)
                            for hh, jb, pb in heads:
                                p = spool.tile([128, 2, 512], F32R, tag="p",
                                               name=f"p{qg}_{hh}_{kp}")
                                nc.scalar.activation(p[:], scs[hh][:], AF.Exp,
                                                     scale=0.125)
                                for j in range(2):
                                    kv = 2 * kp + j
                                    nc.tensor.matmul(
                                        pv[hh][:],
                                        v_sb[:, kv, hh * (HD + 1):(hh + 1) * (HD + 1)],
                                        p[:, j, :],
                                        start=(kv == 0), stop=(kv == NKV - 1))
                        for hh, jb, pb in heads:
                            r = spool.tile([1, 512], F32, tag="r",
                                           name=f"r{qg}_{hh}")
                            nc.vector.reciprocal(r[:], pv[hh][HD:HD + 1, :])
                            rb = spool.tile([64, 512], F32, tag="rb", bufs=2,
                                            name=f"rb{qg}_{hh}")
                            nc.gpsimd.partition_broadcast(rb[:], r[:])
                            yn[hh] = spool.tile([64, 512], F32R, tag="yn", bufs=4,
                                                name=f"yn{qg}_{hh}")
                            nc.vector.tensor_mul(
                                yn[hh][:], pv[hh][0:HD, :], rb[:])

                    head_attn(0, 0, 0, paired_with=(1, 0, 64))
                    head_attn(2, 1, 0)

                    # projection: out[q, f] += y_h[q, :] @ wpT[:, h, f]
                    for f in range(2):
                        fw = 512 if f == 0 else E - 512
                        fsl = slice(f * 512, f * 512 + fw)
                        for qb in range(4):
                            pp = bpsum.tile([128, fw], F32, tag="acc", bufs=2,
                                            name=f"pp{qg}_{f}_{qb}")
                            for h in range(NH):
                                nc.tensor.matmul(
                                    pp[:], yn[h][:, qb * 128:(qb + 1) * 128],
                                    wpT_sb[:, h, fsl],
                                    start=(h == 0), stop=(h == NH - 1))
                            ost = spool.tile([128, fw], F32, tag="ost", bufs=3,
                                             name=f"ost{qg}_{f}_{qb}")
                            nc.vector.tensor_copy(ost[:], pp[:])
                            nc.sync.dma_start(
                                out[qg * 512 + qb * 128:qg * 512 + (qb + 1) * 128,
                                    fsl], ost[:])

    nc.finalize()
    return nc


def host_prep(x, w_qkv, b_qkv, w_proj, b_proj, n_tokens=N):
    """Build per-core input maps + the host-side combine closure."""
    x = np.asarray(x, np.float32)
    w_qkv = np.asarray(w_qkv, np.float32)
    b_qkv = np.asarray(b_qkv, np.float32)
    w_proj = np.asarray(w_proj, np.float32)
    b_proj = np.asarray(b_proj, np.float32)

    xT = [np.ascontiguousarray(x[b].T) for b in range(B)]  # [E, N]

    in_maps = []
    for c in range(8):
        b, g = divmod(c, M_GROUPS)
        base = g * NH * 3 * HD  # row offset of this group in w_qkv (576/group)
        # w_qkv row layout per head h: [h*192, +64)=Q, [+64, +128)=K, [+128, +192)=V
        wq = [w_qkv[base + i * 3 * HD: base + i * 3 * HD + HD] for i in range(NH)]
        wk = [w_qkv[base + i * 3 * HD + HD: base + i * 3 * HD + 2 * HD]
              for i in range(NH)]
        wv = [w_qkv[base + i * 3 * HD + 2 * HD: base + i * 3 * HD + 3 * HD]
              for i in range(NH)]
        bqv = [b_qkv[base + i * 3 * HD: base + i * 3 * HD + HD] for i in range(NH)]
        # m-tiles: m0=[Q0;Q1], m1=[K0;K1], m2=[Q2;K2]  (psum partition layout)
        wqkT = np.concatenate(
            [wq[0], wq[1], wk[0], wk[1], wq[2], wk[2]], axis=0).T  # [E, 384]
        wvT = np.concatenate(wv, axis=0).T  # [E, 192]
        wvT = np.concatenate([wvT, np.zeros((E, GDP - GD), np.float32)], axis=1)
        bq = np.zeros((2, 128), np.float32)
        bq[0, 0:HD] = bqv[0]
        bq[0, HD:2 * HD] = bqv[1]
        bq[1, 0:HD] = bqv[2]
        # wpT[d, h, f] = w_proj[f, g*192 + h*64 + d]
        wp = w_proj[:, g * GD:(g + 1) * GD]  # [768, 192]
        wpT = np.ascontiguousarray(
            wp.T.reshape(NH, HD, E).transpose(1, 0, 2))  # [64, 3, 768]
        in_maps.append({
            "xT": np.ascontiguousarray(xT[b]),
            "wqkT": np.ascontiguousarray(wqkT),
            "wvT": np.ascontiguousarray(wvT),
            "bq": bq,
            "wpT": wpT,
        })

    # fold V bias through the projection into the output bias
    bv_all = np.concatenate(
        [b_qkv[h * 3 * HD + 2 * HD: (h + 1) * 3 * HD] for h in range(H)])  # [768]
    b_eff = b_proj + w_proj @ bv_all

    def combine(results):
        out = np.empty((B, n_tokens, E), np.float32)
        for b in range(B):
            acc = results[b * M_GROUPS]["out"].astype(np.float32)
            for g in range(1, M_GROUPS):
                acc = acc + results[b * M_GROUPS + g]["out"]
            out[b] = acc + b_eff
        return out

    return in_maps, combine


_NC_CACHE = {}


def kernel(x, w_qkv, b_qkv, w_proj, b_proj):
    if "nc" not in _NC_CACHE:
        _NC_CACHE["nc"] = build_nc()
    nc = _NC_CACHE["nc"]
    in_maps, combine = host_prep(x, w_qkv, b_qkv, w_proj, b_proj)
    res = run_bass_kernel_spmd(nc, in_maps, core_ids=list(range(8)))
    return combine(res.results)


if __name__ == "__main__":
    rng = np.random.default_rng(0)
    inputs = {
        "x": rng.normal(size=(B, N, E)).astype(np.float32),
        "w_qkv": (rng.normal(size=(3 * E, E)) * 0.02).astype(np.float32),
        "b_qkv": (rng.normal(size=(3 * E,)) * 0.02).astype(np.float32),
        "w_proj": (rng.normal(size=(E, E)) * 0.02).astype(np.float32),
        "b_proj": (rng.normal(size=(E,)) * 0.02).astype(np.float32),
    }
    out = kernel(**inputs)
    print("out", out.shape, out.dtype, float(np.abs(out).mean()))


# revision 25
# speedup vs baseline: 1.0626x; 1.0626x over previous
"""Multi-head attention Bass kernel for Trainium2 (8 NeuronCores).

Problem: B=2, N=4096, E=768, H=12 heads of dim 64 (nn_MultiHeadAttention).
Sharding: 2 batches x 4 head-groups (3 heads each) = 8 cores. Each core:
  - QKV projection for its 3 heads (x pre-transposed on host to [E, N])
  - flash-style attention with transposed scores P[kv, q] (no max subtraction:
    scores are tightly bounded ~N(0, 0.3^2) for this problem's scale)
  - softmax denominators via a ones-column appended to V in the P@V matmul
  - output projection against its 192 w_proj rows -> partial [N, 768]
Host: sums the 4 partials per batch and adds the (bias-folded) b_proj.

Bias handling (exact algebra, no approximation):
  - K bias drops out of softmax (adds a per-query constant to scores).
  - V bias commutes through P@V normalization; bv @ w_proj.T folds into b_proj.
  - Q bias is applied on device (per-partition bias in the QKV->SBUF copy).
"""

import sys

sys.path.insert(0, "/opt/trn_rl_repo")

import numpy as np

import concourse.bass as bass  # noqa: E402
import concourse.mybir as mybir  # noqa: E402
import concourse.tile as tile  # noqa: E402
from concourse import bacc  # noqa: E402
from concourse.bass_utils import run_bass_kernel_spmd  # noqa: E402

F32 = mybir.dt.float32
F32R = mybir.dt.float32r


def _r(ap):
    """Bitcast an fp32 AP to float32r for full-rate PE matmuls."""
    return ap.bitcast(F32R)
AF = mybir.ActivationFunctionType

B, N, E = 2, 4096, 768
H, HD = 12, 64
NH = 3          # heads per core
M_GROUPS = 4    # head groups (tensor parallel)
GD = NH * HD    # 192 y-dims per core
GDP = 256       # V matmul moving dim padded to 256 (f32r full-rate needs >=256)
QKDIM = 2 * NH * HD  # 384 qk output dims per core


def build_nc(n_tokens=N, num_devices=8):
    """Build the per-core Bass module (SPMD: same program, different data)."""
    n = n_tokens
    NQG = n // 512          # q groups of 512
    NKV = n // 128          # kv blocks of 128
    KE = E // 128           # contraction tiles over E

    nc = bacc.Bacc("TRN2", target_bir_lowering=False, debug=False,
                   num_devices=num_devices)

    xT = nc.dram_tensor("xT", [E, n], F32R, kind="ExternalInput")
    wqkT = nc.dram_tensor("wqkT", [E, QKDIM], F32R, kind="ExternalInput")
    wvT = nc.dram_tensor("wvT", [E, GDP], F32R, kind="ExternalInput")
    bq = nc.dram_tensor("bq", [2, 128], F32, kind="ExternalInput")
    wpT = nc.dram_tensor("wpT", [HD, NH, E], F32R, kind="ExternalInput")
    out = nc.dram_tensor("out", [n, E], F32, kind="ExternalOutput")

    with tile.TileContext(nc) as tc:
        with (
            tc.tile_pool(name="perm", bufs=1) as perm,
            tc.tile_pool(name="wpool", bufs=1) as wpool,
        ):
            # Persistent SBUF tensors
            # qk_sb[:, j, 0:n] = Q.T area, [:, j, n:2n] = K.T area.
            # j=0: head0 on partitions 0:64, head1 on 64:128; j=1: head2 on 0:64.
            qk_sb = perm.tile([128, 2, 2 * n], F32R)
            # V (+ ones col per head) in [kv, d] layout: per kv-block of 128
            # tokens, 3 heads x (64 dims + ones col).
            v_sb = perm.tile([128, NKV, NH * (HD + 1)], F32R)

            wqkT_sb = wpool.tile([128, KE, QKDIM], F32R)
            wvT_sb = wpool.tile([128, KE, GDP], F32R)
            wpT_sb = wpool.tile([64, NH, E], F32R)
            bq_sb = wpool.tile([128, 2], F32)

            nc.sync.dma_start(wqkT_sb[:], wqkT.rearrange("(a p) c -> p a c", p=128))
            nc.sync.dma_start(wvT_sb[:], wvT.rearrange("(a p) c -> p a c", p=128))
            nc.sync.dma_start(wpT_sb[:], wpT[:])
            nc.sync.dma_start(bq_sb[:], bq.rearrange("a p -> p a"))

            # ones columns for the softmax-denominator trick
            ones_view = v_sb.rearrange("p a (h c) -> p a h c", c=HD + 1)[:, :, :, HD:]
            nc.vector.memset(ones_view.bitcast(F32), 1.0)

            # ---- One PSUM budget for everything (8 banks): tag "a" (2
            # banks) is time-shared by QKV-projection tiles and the output-
            # projection accumulators; "sc" 4 banks; "pv" 2 banks. This lets
            # the scheduler overlap the QKV projection with attention. ----
            with (
                tc.tile_pool(name="apsum", bufs=1, space="PSUM") as apsum,
                tc.tile_pool(name="bpsum", bufs=1, space="PSUM") as bpsum,
                tc.tile_pool(name="xpool", bufs=13) as xpool,
                tc.tile_pool(name="spool", bufs=3) as spool,
            ):
                for ng in range(NQG):
                    xts = []
                    for k in range(KE):
                        xt = xpool.tile([128, 512], F32R, tag="xt",
                                        name=f"xt{ng}_{k}")
                        nc.sync.dma_start(xt[:], xT[k * 128:(k + 1) * 128,
                                                    ng * 512:(ng + 1) * 512])
                        xts.append(xt)
                    qs = slice(ng * 512, (ng + 1) * 512)
                    ks = slice(n + ng * 512, n + (ng + 1) * 512)
                    for m in range(3):
                        psq = apsum.tile([128, 512], F32, tag="a", bufs=2,
                                         name=f"psq{ng}_{m}")
                        for k in range(KE):
                            nc.tensor.matmul(psq[:],
                                             wqkT_sb[:, k, m * 128:(m + 1) * 128],
                                             xts[k][:], start=(k == 0),
                                             stop=(k == KE - 1))
                        if m == 0:  # Q head0/1 + bias
                            nc.vector.tensor_scalar_add(qk_sb[:, 0, qs], psq[:],
                                                        bq_sb[:, 0:1])
                        elif m == 1:  # K head0/1
                            nc.vector.tensor_copy(qk_sb[:, 0, ks], psq[:])
                        else:  # m2 = [Q head2 ; K head2]
                            nc.vector.tensor_scalar_add(qk_sb[0:64, 1, qs],
                                                        psq[0:64, :],
                                                        bq_sb[0:64, 1:2])
                            # K head2 must live on partitions 0:64 (same as
                            # its Q). DMA can't read PSUM, so stage in SBUF
                            # then do a partition-shifting SBUF->SBUF DMA.
                            k2st = xpool.tile([128, 512], F32R, tag="k2st",
                                              bufs=2, name=f"k2st{ng}")
                            nc.vector.tensor_copy(k2st[64:128, :],
                                                  psq[64:128, :])
                            nc.sync.dma_start(qk_sb[0:64, 1, ks],
                                              k2st[64:128, :])
                    # V projection: 2 kv-blocks per 1-bank tile, j-outer
                    # so each bank hosts one accumulation group at a time
                    for vj in range(2):
                        psv = apsum.tile([128, 2, GDP], F32, tag="a", bufs=2,
                                         name=f"psv{ng}_{vj}")
                        for j in range(2):
                            jj = 2 * vj + j
                            for k in range(KE):
                                nc.tensor.matmul(
                                    psv[:, j, :],
                                    xts[k][:, jj * 128:(jj + 1) * 128],
                                    wvT_sb[:, k, :], start=(k == 0),
                                    stop=(k == KE - 1))
                        dst = v_sb[:, ng * 4 + 2 * vj:ng * 4 + 2 * vj + 2,
                                   :].rearrange(
                            "p a (h c) -> p a h c", c=HD + 1)[:, :, :, 0:HD]
                        src_ap = psv[:, :, 0:GD].rearrange(
                            "p a (h c) -> p a h c", c=HD)
                        nc.vector.tensor_copy(dst, src_ap)

                # ---- Stage B+C: software-pipelined attention ----
                # job list: (qg, headset, kp). Scores for job i+1 are emitted
                # between exp(i) and pv(i) so ACT never waits on PE.
                HEADSETS = [[(0, 0, 0), (1, 0, 64)], [(2, 1, 0)]]
                jobs = [(qg, hs, kp)
                        for qg in range(NQG)
                        for hs in range(2)
                        for kp in range(NKV // 2)]
                pvp_tiles = {}
                yn = {}

                def emit_scores(qg, hs, kp):
                    qsl = slice(qg * 512, (qg + 1) * 512)
                    scs = {}
                    for hh, jb, pb in HEADSETS[hs]:
                        scs[hh] = bpsum.tile([128, 2, 512], F32, tag="sc",
                                             bufs=2, name=f"sc{qg}_{hh}_{kp}")
                    # j-outer / head-inner: consecutive matmuls use disjoint
                    # PE row groups (partitions 0:64 vs 64:128) and overlap
                    # in the 128x128 array on hardware.
                    for j in range(2):
                        kv = 2 * kp + j
                        for hh, jb, pb in HEADSETS[hs]:
                            lhs = qk_sb[pb:pb + 64, jb,
                                        n + kv * 128:n + (kv + 1) * 128]
                            rhs = qk_sb[pb:pb + 64, jb, qsl]
                            nc.tensor.matmul(scs[hh][:, j, :], lhs, rhs,
                                             start=True, stop=True)
                    return scs

                def emit_norm(qg, hs):
                    for hh, jb, pb in HEADSETS[hs]:
                        pvh = pvp_tiles[(qg, hs)][1][hh]
                        r = spool.tile([1, 512], F32, tag="r",
                                       name=f"r{qg}_{hh}")
                        nc.vector.reciprocal(r[:], pvh[HD:HD + 1, :])
                        rb = spool.tile([64, 512], F32, tag="rb", bufs=2,
                                        name=f"rb{qg}_{hh}")
                        nc.gpsimd.partition_broadcast(rb[:], r[:])
                        yn[hh] = spool.tile([64, 512], F32R, tag="yn", bufs=6,
                                            name=f"yn{qg}_{hh}")
                        nc.vector.tensor_mul(yn[hh][:], pvh[0:HD, :], rb[:])

                def emit_proj(qg):
                    for f in range(2):
                        fw = 512 if f == 0 else E - 512
                        fsl = slice(f * 512, f * 512 + fw)
                        for qb in range(4):
                            pp = apsum.tile([128, fw], F32, tag="a", bufs=2,
                                            name=f"pp{qg}_{f}_{qb}")
                            for h in range(NH):
                                nc.tensor.matmul(
                                    pp[:], yn[h][:, qb * 128:(qb + 1) * 128],
                                    wpT_sb[:, h, fsl],
                                    start=(h == 0), stop=(h == NH - 1))
                            ost = spool.tile([128, fw], F32, tag="ost", bufs=4,
                                             name=f"ost{qg}_{f}_{qb}")
                            nc.vector.tensor_copy(ost[:], pp[:])
                            nc.sync.dma_start(
                                out[qg * 512 + qb * 128:
                                    qg * 512 + (qb + 1) * 128, fsl], ost[:])

                scs_cur = emit_scores(*jobs[0])
                for idx, (qg, hs, kp) in enumerate(jobs):
                    if kp == 0:
                        pvp = bpsum.tile([HD + 1, 2, 512], F32, tag="pv",
                                         bufs=1, name=f"pv{qg}_{hs}")
                        pvp_tiles[(qg, hs)] = (
                            pvp, {hh: pvp[:, i, :] for i, (hh, _, _) in
                                  enumerate(HEADSETS[hs])})
                    pvs = pvp_tiles[(qg, hs)][1]
                    ps = {}
                    for hh, jb, pb in HEADSETS[hs]:
                        p = spool.tile([128, 2, 512], F32R, tag="p", bufs=6,
                                       name=f"p{qg}_{hh}_{kp}")
                        nc.scalar.activation(p[:], scs_cur[hh][:], AF.Exp,
                                             scale=0.125)
                        ps[hh] = p
                    scs_next = (emit_scores(*jobs[idx + 1])
                                if idx + 1 < len(jobs) else None)
                    for hh, jb, pb in HEADSETS[hs]:
                        for j in range(2):
                            kv = 2 * kp + j
                            nc.tensor.matmul(
                                pvs[hh],
                                v_sb[:, kv, hh * (HD + 1):(hh + 1) * (HD + 1)],
                                ps[hh][:, j, :],
                                start=(kv == 0), stop=(kv == NKV - 1))
                    if kp == NKV // 2 - 1:
                        emit_norm(qg, hs)
                        if hs == 1:
                            emit_proj(qg)
                    scs_cur = scs_next

    nc.finalize()
    return nc


def host_prep(x, w_qkv, b_qkv, w_proj, b_proj, n_tokens=N):
    """Build per-core input maps + the host-side combine closure."""
    x = np.asarray(x, np.float32)
    w_qkv = np.asarray(w_qkv, np.float32)
    b_qkv = np.asarray(b_qkv, np.float32)
    w_proj = np.asarray(w_proj, np.float32)
    b_proj = np.asarray(b_proj, np.float32)

    xT = [np.ascontiguousarray(x[b].T) for b in range(B)]  # [E, N]

    in_maps = []
    for c in range(8):
        b, g = divmod(c, M_GROUPS)
        base = g * NH * 3 * HD  # row offset of this group in w_qkv (576/group)
        # w_qkv row layout per head h: [h*192, +64)=Q, [+64, +128)=K, [+128, +192)=V
        wq = [w_qkv[base + i * 3 * HD: base + i * 3 * HD + HD] for i in range(NH)]
        wk = [w_qkv[base + i * 3 * HD + HD: base + i * 3 * HD + 2 * HD]
              for i in range(NH)]
        wv = [w_qkv[base + i * 3 * HD + 2 * HD: base + i * 3 * HD + 3 * HD]
              for i in range(NH)]
        bqv = [b_qkv[base + i * 3 * HD: base + i * 3 * HD + HD] for i in range(NH)]
        # m-tiles: m0=[Q0;Q1], m1=[K0;K1], m2=[Q2;K2]  (psum partition layout)
        wqkT = np.concatenate(
            [wq[0], wq[1], wk[0], wk[1], wq[2], wk[2]], axis=0).T  # [E, 384]
        wvT = np.concatenate(wv, axis=0).T  # [E, 192]
        wvT = np.concatenate([wvT, np.zeros((E, GDP - GD), np.float32)], axis=1)
        bq = np.zeros((2, 128), np.float32)
        bq[0, 0:HD] = bqv[0]
        bq[0, HD:2 * HD] = bqv[1]
        bq[1, 0:HD] = bqv[2]
        # wpT[d, h, f] = w_proj[f, g*192 + h*64 + d]
        wp = w_proj[:, g * GD:(g + 1) * GD]  # [768, 192]
        wpT = np.ascontiguousarray(
            wp.T.reshape(NH, HD, E).transpose(1, 0, 2))  # [64, 3, 768]
        in_maps.append({
            "xT": np.ascontiguousarray(xT[b]),
            "wqkT": np.ascontiguousarray(wqkT),
            "wvT": np.ascontiguousarray(wvT),
            "bq": bq,
            "wpT": wpT,
        })

    # fold V bias through the projection into the output bias
    bv_all = np.concatenate(
        [b_qkv[h * 3 * HD + 2 * HD: (h + 1) * 3 * HD] for h in range(H)])  # [768]
    b_eff = b_proj + w_proj @ bv_all

    def combine(results):
        out = np.empty((B, n_tokens, E), np.float32)
        for b in range(B):
            acc = results[b * M_GROUPS]["out"].astype(np.float32)
            for g in range(1, M_GROUPS):
                acc = acc + results[b * M_GROUPS + g]["out"]
            out[b] = acc + b_eff
        return out

    return in_maps, combine


_NC_CACHE = {}


def kernel(x, w_qkv, b_qkv, w_proj, b_proj):
    if "nc" not in _NC_CACHE:
        _NC_CACHE["nc"] = build_nc()
    nc = _NC_CACHE["nc"]
    in_maps, combine = host_prep(x, w_qkv, b_qkv, w_proj, b_proj)
    res = run_bass_kernel_spmd(nc, in_maps, core_ids=list(range(8)))
    return combine(res.results)


if __name__ == "__main__":
    rng = np.random.default_rng(0)
    inputs = {
        "x": rng.normal(size=(B, N, E)).astype(np.float32),
        "w_qkv": (rng.normal(size=(3 * E, E)) * 0.02).astype(np.float32),
        "b_qkv": (rng.normal(size=(3 * E,)) * 0.02).astype(np.float32),
        "w_proj": (rng.normal(size=(E, E)) * 0.02).astype(np.float32),
        "b_proj": (rng.normal(size=(E,)) * 0.02).astype(np.float32),
    }
    out = kernel(**inputs)
    print("out", out.shape, out.dtype, float(np.abs(out).mean()))


# revision 27
# speedup vs baseline: 1.0830x; 1.0192x over previous
"""Multi-head attention Bass kernel for Trainium2 (8 NeuronCores).

Problem: B=2, N=4096, E=768, H=12 heads of dim 64 (nn_MultiHeadAttention).
Sharding: 2 batches x 4 head-groups (3 heads each) = 8 cores. Each core:
  - QKV projection for its 3 heads (x pre-transposed on host to [E, N])
  - flash-style attention with transposed scores P[kv, q] (no max subtraction:
    scores are tightly bounded ~N(0, 0.3^2) for this problem's scale)
  - softmax denominators via a ones-column appended to V in the P@V matmul
  - output projection against its 192 w_proj rows -> partial [N, 768]
Host: sums the 4 partials per batch and adds the (bias-folded) b_proj.

Bias handling (exact algebra, no approximation):
  - K bias drops out of softmax (adds a per-query constant to scores).
  - V bias commutes through P@V normalization; bv @ w_proj.T folds into b_proj.
  - Q bias is applied on device (per-partition bias in the QKV->SBUF copy).
"""

import sys

sys.path.insert(0, "/opt/trn_rl_repo")

import numpy as np

import concourse.bass as bass  # noqa: E402
import concourse.mybir as mybir  # noqa: E402
import concourse.tile as tile  # noqa: E402
from concourse import bacc  # noqa: E402
from concourse.bass_utils import run_bass_kernel_spmd  # noqa: E402

F32 = mybir.dt.float32
F32R = mybir.dt.float32r


def _r(ap):
    """Bitcast an fp32 AP to float32r for full-rate PE matmuls."""
    return ap.bitcast(F32R)
AF = mybir.ActivationFunctionType

B, N, E = 2, 4096, 768
H, HD = 12, 64
NH = 3          # heads per core
M_GROUPS = 4    # head groups (tensor parallel)
GD = NH * HD    # 192 y-dims per core
GDP = 256       # V matmul moving dim padded to 256 (f32r full-rate needs >=256)
QKDIM = 2 * NH * HD  # 384 qk output dims per core


def build_nc(n_tokens=N, num_devices=8):
    """Build the per-core Bass module (SPMD: same program, different data)."""
    n = n_tokens
    NQG = n // 512          # q groups of 512
    NKV = n // 128          # kv blocks of 128
    KE = E // 128           # contraction tiles over E

    nc = bacc.Bacc("TRN2", target_bir_lowering=False, debug=False,
                   num_devices=num_devices)

    xT = nc.dram_tensor("xT", [E, n], F32R, kind="ExternalInput")
    wqkT = nc.dram_tensor("wqkT", [E, QKDIM], F32R, kind="ExternalInput")
    wvT = nc.dram_tensor("wvT", [E, GDP], F32R, kind="ExternalInput")
    bq = nc.dram_tensor("bq", [2, 128], F32, kind="ExternalInput")
    wpT = nc.dram_tensor("wpT", [HD, NH, E], F32R, kind="ExternalInput")
    out = nc.dram_tensor("out", [n, E], F32, kind="ExternalOutput")

    with tile.TileContext(nc) as tc:
        with (
            tc.tile_pool(name="perm", bufs=1) as perm,
            tc.tile_pool(name="wpool", bufs=1) as wpool,
        ):
            # Persistent SBUF tensors
            # qk_sb[:, j, 0:n] = Q.T area, [:, j, n:2n] = K.T area.
            # j=0: head0 on partitions 0:64, head1 on 64:128; j=1: head2 on 0:64.
            qk_sb = perm.tile([128, 2, 2 * n], F32R)
            # V (+ ones col per head) in [kv, d] layout: per kv-block of 128
            # tokens, 3 heads x (64 dims + ones col).
            v_sb = perm.tile([128, NKV, NH * (HD + 1)], F32R)

            wqkT_sb = wpool.tile([128, KE, QKDIM], F32R)
            wvT_sb = wpool.tile([128, KE, GDP], F32R)
            wpT_sb = wpool.tile([64, NH, E], F32R)
            bq_sb = wpool.tile([128, 2], F32)

            nc.sync.dma_start(wqkT_sb[:], wqkT.rearrange("(a p) c -> p a c", p=128))
            nc.sync.dma_start(wvT_sb[:], wvT.rearrange("(a p) c -> p a c", p=128))
            nc.sync.dma_start(wpT_sb[:], wpT[:])
            nc.sync.dma_start(bq_sb[:], bq.rearrange("a p -> p a"))

            # ones columns for the softmax-denominator trick
            ones_view = v_sb.rearrange("p a (h c) -> p a h c", c=HD + 1)[:, :, :, HD:]
            nc.vector.memset(ones_view.bitcast(F32), 1.0)

            # ---- One PSUM budget for everything (8 banks): tag "a" (2
            # banks) is time-shared by QKV-projection tiles and the output-
            # projection accumulators; "sc" 4 banks; "pv" 2 banks. This lets
            # the scheduler overlap the QKV projection with attention. ----
            with (
                tc.tile_pool(name="apsum", bufs=1, space="PSUM") as apsum,
                tc.tile_pool(name="bpsum", bufs=1, space="PSUM") as bpsum,
                tc.tile_pool(name="xpool", bufs=13) as xpool,
                tc.tile_pool(name="spool", bufs=3) as spool,
            ):
                for ng in range(NQG):
                    xts = []
                    for k in range(KE):
                        xt = xpool.tile([128, 512], F32R, tag="xt",
                                        name=f"xt{ng}_{k}")
                        nc.sync.dma_start(xt[:], xT[k * 128:(k + 1) * 128,
                                                    ng * 512:(ng + 1) * 512])
                        xts.append(xt)
                    qs = slice(ng * 512, (ng + 1) * 512)
                    ks = slice(n + ng * 512, n + (ng + 1) * 512)
                    for m in range(3):
                        psq = apsum.tile([128, 512], F32, tag="a", bufs=2,
                                         name=f"psq{ng}_{m}")
                        for k in range(KE):
                            nc.tensor.matmul(psq[:],
                                             wqkT_sb[:, k, m * 128:(m + 1) * 128],
                                             xts[k][:], start=(k == 0),
                                             stop=(k == KE - 1))
                        if m == 0:  # Q head0/1 + bias
                            nc.vector.tensor_scalar_add(qk_sb[:, 0, qs], psq[:],
                                                        bq_sb[:, 0:1])
                        elif m == 1:  # K head0/1
                            nc.vector.tensor_copy(qk_sb[:, 0, ks], psq[:])
                        else:  # m2 = [Q head2 ; K head2]
                            nc.vector.tensor_scalar_add(qk_sb[0:64, 1, qs],
                                                        psq[0:64, :],
                                                        bq_sb[0:64, 1:2])
                            # K head2 must live on partitions 0:64 (same as
                            # its Q). DMA can't read PSUM, so stage in SBUF
                            # then do a partition-shifting SBUF->SBUF DMA.
                            k2st = xpool.tile([128, 512], F32R, tag="k2st",
                                              bufs=2, name=f"k2st{ng}")
                            nc.vector.tensor_copy(k2st[64:128, :],
                                                  psq[64:128, :])
                            nc.sync.dma_start(qk_sb[0:64, 1, ks],
                                              k2st[64:128, :])
                    # V projection: 2 kv-blocks per 1-bank tile, j-outer
                    # so each bank hosts one accumulation group at a time
                    for vj in range(2):
                        psv = apsum.tile([128, 2, GDP], F32, tag="a", bufs=2,
                                         name=f"psv{ng}_{vj}")
                        for j in range(2):
                            jj = 2 * vj + j
                            for k in range(KE):
                                nc.tensor.matmul(
                                    psv[:, j, :],
                                    xts[k][:, jj * 128:(jj + 1) * 128],
                                    wvT_sb[:, k, :], start=(k == 0),
                                    stop=(k == KE - 1))
                        dst = v_sb[:, ng * 4 + 2 * vj:ng * 4 + 2 * vj + 2,
                                   :].rearrange(
                            "p a (h c) -> p a h c", c=HD + 1)[:, :, :, 0:HD]
                        src_ap = psv[:, :, 0:GD].rearrange(
                            "p a (h c) -> p a h c", c=HD)
                        nc.vector.tensor_copy(dst, src_ap)

                # ---- Stage B+C: software-pipelined attention ----
                # Single-head jobs (qg, h, kp), h0/h1 interleaved per kp so
                # consecutive scores matmuls hit disjoint PE row groups.
                # Scores are emitted at pipeline depth 2 (one full exp of
                # slack) so ACT never waits on PE.
                HEADS = {0: (0, 0), 1: (0, 64), 2: (1, 0)}  # h -> (jblk, pbase)
                jobs = []
                for qg in range(NQG):
                    for kp in range(NKV // 2):
                        jobs += [(qg, 0, kp), (qg, 1, kp)]
                    jobs += [(qg, 2, kp) for kp in range(NKV // 2)]
                pvp_tiles = {}
                yn = {}

                def emit_scores(qg, h, kp):
                    qsl = slice(qg * 512, (qg + 1) * 512)
                    jb, pb = HEADS[h]
                    sc = bpsum.tile([128, 2, 512], F32, tag="sc",
                                    bufs=2, name=f"sc{qg}_{h}_{kp}")
                    for j in range(2):
                        kv = 2 * kp + j
                        lhs = qk_sb[pb:pb + 64, jb,
                                    n + kv * 128:n + (kv + 1) * 128]
                        rhs = qk_sb[pb:pb + 64, jb, qsl]
                        nc.tensor.matmul(sc[:, j, :], lhs, rhs,
                                         start=True, stop=True)
                    return sc

                def emit_norm(qg, hh):
                    pvh = pvp_tiles[(qg, hh)]
                    r = spool.tile([1, 512], F32, tag="r",
                                   name=f"r{qg}_{hh}")
                    nc.vector.reciprocal(r[:], pvh[HD:HD + 1, :])
                    rb = spool.tile([64, 512], F32, tag="rb", bufs=2,
                                    name=f"rb{qg}_{hh}")
                    nc.gpsimd.partition_broadcast(rb[:], r[:])
                    yn[hh] = spool.tile([64, 512], F32R, tag="yn", bufs=6,
                                        name=f"yn{qg}_{hh}")
                    nc.vector.tensor_mul(yn[hh][:], pvh[0:HD, :], rb[:])

                def emit_proj(qg):
                    for f in range(2):
                        fw = 512 if f == 0 else E - 512
                        fsl = slice(f * 512, f * 512 + fw)
                        for qb in range(4):
                            pp = apsum.tile([128, fw], F32, tag="a", bufs=2,
                                            name=f"pp{qg}_{f}_{qb}")
                            for h in range(NH):
                                nc.tensor.matmul(
                                    pp[:], yn[h][:, qb * 128:(qb + 1) * 128],
                                    wpT_sb[:, h, fsl],
                                    start=(h == 0), stop=(h == NH - 1))
                            ost = spool.tile([128, fw], F32, tag="ost", bufs=4,
                                             name=f"ost{qg}_{f}_{qb}")
                            nc.vector.tensor_copy(ost[:], pp[:])
                            nc.sync.dma_start(
                                out[qg * 512 + qb * 128:
                                    qg * 512 + (qb + 1) * 128, fsl], ost[:])

                pending = [emit_scores(*jobs[0]), emit_scores(*jobs[1])]
                for idx, (qg, hh, kp) in enumerate(jobs):
                    if kp == 0:
                        if hh == 0:  # one 2-bank tensor for the h0/h1 pair
                            pvp = bpsum.tile([HD + 1, 2, 512], F32, tag="pv",
                                             bufs=1, name=f"pv{qg}_01")
                            pvp_tiles[(qg, 0)] = pvp[:, 0, :]
                            pvp_tiles[(qg, 1)] = pvp[:, 1, :]
                        elif hh == 2:
                            pv2 = bpsum.tile([HD + 1, 2, 512], F32, tag="pv",
                                             bufs=1, name=f"pv{qg}_2")
                            pvp_tiles[(qg, 2)] = pv2[:, 0, :]
                    sc = pending.pop(0)
                    p = spool.tile([128, 2, 512], F32R, tag="p", bufs=6,
                                   name=f"p{qg}_{hh}_{kp}")
                    nc.scalar.activation(p[:], sc[:], AF.Exp, scale=0.125)
                    if idx + 2 < len(jobs):
                        pending.append(emit_scores(*jobs[idx + 2]))
                    for j in range(2):
                        kv = 2 * kp + j
                        nc.tensor.matmul(
                            pvp_tiles[(qg, hh)],
                            v_sb[:, kv, hh * (HD + 1):(hh + 1) * (HD + 1)],
                            p[:, j, :],
                            start=(kv == 0), stop=(kv == NKV - 1))
                    if kp == NKV // 2 - 1:
                        emit_norm(qg, hh)
                        if hh == 2:
                            emit_proj(qg)

    nc.finalize()
    return nc


def host_prep(x, w_qkv, b_qkv, w_proj, b_proj, n_tokens=N):
    """Build per-core input maps + the host-side combine closure."""
    x = np.asarray(x, np.float32)
    w_qkv = np.asarray(w_qkv, np.float32)
    b_qkv = np.asarray(b_qkv, np.float32)
    w_proj = np.asarray(w_proj, np.float32)
    b_proj = np.asarray(b_proj, np.float32)

    xT = [np.ascontiguousarray(x[b].T) for b in range(B)]  # [E, N]

    in_maps = []
    for c in range(8):
        b, g = divmod(c, M_GROUPS)
        base = g * NH * 3 * HD  # row offset of this group in w_qkv (576/group)
        # w_qkv row layout per head h: [h*192, +64)=Q, [+64, +128)=K, [+128, +192)=V
        wq = [w_qkv[base + i * 3 * HD: base + i * 3 * HD + HD] for i in range(NH)]
        wk = [w_qkv[base + i * 3 * HD + HD: base + i * 3 * HD + 2 * HD]
              for i in range(NH)]
        wv = [w_qkv[base + i * 3 * HD + 2 * HD: base + i * 3 * HD + 3 * HD]
              for i in range(NH)]
        bqv = [b_qkv[base + i * 3 * HD: base + i * 3 * HD + HD] for i in range(NH)]
        # m-tiles: m0=[Q0;Q1], m1=[K0;K1], m2=[Q2;K2]  (psum partition layout)
        wqkT = np.concatenate(
            [wq[0], wq[1], wk[0], wk[1], wq[2], wk[2]], axis=0).T  # [E, 384]
        wvT = np.concatenate(wv, axis=0).T  # [E, 192]
        wvT = np.concatenate([wvT, np.zeros((E, GDP - GD), np.float32)], axis=1)
        bq = np.zeros((2, 128), np.float32)
        bq[0, 0:HD] = bqv[0]
        bq[0, HD:2 * HD] = bqv[1]
        bq[1, 0:HD] = bqv[2]
        # wpT[d, h, f] = w_proj[f, g*192 + h*64 + d]
        wp = w_proj[:, g * GD:(g + 1) * GD]  # [768, 192]
        wpT = np.ascontiguousarray(
            wp.T.reshape(NH, HD, E).transpose(1, 0, 2))  # [64, 3, 768]
        in_maps.append({
            "xT": np.ascontiguousarray(xT[b]),
            "wqkT": np.ascontiguousarray(wqkT),
            "wvT": np.ascontiguousarray(wvT),
            "bq": bq,
            "wpT": wpT,
        })

    # fold V bias through the projection into the output bias
    bv_all = np.concatenate(
        [b_qkv[h * 3 * HD + 2 * HD: (h + 1) * 3 * HD] for h in range(H)])  # [768]
    b_eff = b_proj + w_proj @ bv_all

    def combine(results):
        out = np.empty((B, n_tokens, E), np.float32)
        for b in range(B):
            acc = results[b * M_GROUPS]["out"].astype(np.float32)
            for g in range(1, M_GROUPS):
                acc = acc + results[b * M_GROUPS + g]["out"]
            out[b] = acc + b_eff
        return out

    return in_maps, combine


_NC_CACHE = {}


def kernel(x, w_qkv, b_qkv, w_proj, b_proj):
    if "nc" not in _NC_CACHE:
        _NC_CACHE["nc"] = build_nc()
    nc = _NC_CACHE["nc"]
    in_maps, combine = host_prep(x, w_qkv, b_qkv, w_proj, b_proj)
    res = run_bass_kernel_spmd(nc, in_maps, core_ids=list(range(8)))
    return combine(res.results)


if __name__ == "__main__":
    rng = np.random.default_rng(0)
    inputs = {
        "x": rng.normal(size=(B, N, E)).astype(np.float32),
        "w_qkv": (rng.normal(size=(3 * E, E)) * 0.02).astype(np.float32),
        "b_qkv": (rng.normal(size=(3 * E,)) * 0.02).astype(np.float32),
        "w_proj": (rng.normal(size=(E, E)) * 0.02).astype(np.float32),
        "b_proj": (rng.normal(size=(E,)) * 0.02).astype(np.float32),
    }
    out = kernel(**inputs)
    print("out", out.shape, out.dtype, float(np.abs(out).mean()))


# revision 28
# speedup vs baseline: 1.1059x; 1.0211x over previous
"""Multi-head attention Bass kernel for Trainium2 (8 NeuronCores).

Problem: B=2, N=4096, E=768, H=12 heads of dim 64 (nn_MultiHeadAttention).
Sharding: 2 batches x 4 head-groups (3 heads each) = 8 cores. Each core:
  - QKV projection for its 3 heads (x pre-transposed on host to [E, N])
  - flash-style attention with transposed scores P[kv, q] (no max subtraction:
    scores are tightly bounded ~N(0, 0.3^2) for this problem's scale)
  - softmax denominators via a ones-column appended to V in the P@V matmul
  - output projection against its 192 w_proj rows -> partial [N, 768]
Host: sums the 4 partials per batch and adds the (bias-folded) b_proj.

Bias handling (exact algebra, no approximation):
  - K bias drops out of softmax (adds a per-query constant to scores).
  - V bias commutes through P@V normalization; bv @ w_proj.T folds into b_proj.
  - Q bias is applied on device (per-partition bias in the QKV->SBUF copy).
"""

import sys

sys.path.insert(0, "/opt/trn_rl_repo")

import numpy as np

import concourse.bass as bass  # noqa: E402
import concourse.mybir as mybir  # noqa: E402
import concourse.tile as tile  # noqa: E402
from concourse import bacc  # noqa: E402
from concourse.bass_utils import run_bass_kernel_spmd  # noqa: E402

F32 = mybir.dt.float32
F32R = mybir.dt.float32r


def _r(ap):
    """Bitcast an fp32 AP to float32r for full-rate PE matmuls."""
    return ap.bitcast(F32R)
AF = mybir.ActivationFunctionType

B, N, E = 2, 4096, 768
H, HD = 12, 64
NH = 3          # heads per core
M_GROUPS = 4    # head groups (tensor parallel)
GD = NH * HD    # 192 y-dims per core
GDP = 256       # V matmul moving dim padded to 256 (f32r full-rate needs >=256)
QKDIM = 2 * NH * HD  # 384 qk output dims per core


def build_nc(n_tokens=N, num_devices=8):
    """Build the per-core Bass module (SPMD: same program, different data)."""
    n = n_tokens
    NQG = n // 512          # q groups of 512
    NKV = n // 128          # kv blocks of 128
    KE = E // 128           # contraction tiles over E

    nc = bacc.Bacc("TRN2", target_bir_lowering=False, debug=False,
                   num_devices=num_devices)

    xT = nc.dram_tensor("xT", [E, n], F32R, kind="ExternalInput")
    wqkT = nc.dram_tensor("wqkT", [E, QKDIM], F32R, kind="ExternalInput")
    wvT = nc.dram_tensor("wvT", [E, GDP], F32R, kind="ExternalInput")
    bq = nc.dram_tensor("bq", [2, 128], F32, kind="ExternalInput")
    wpT = nc.dram_tensor("wpT", [HD, NH, E], F32R, kind="ExternalInput")
    out = nc.dram_tensor("out", [n, E], F32, kind="ExternalOutput")

    with tile.TileContext(nc) as tc:
        with (
            tc.tile_pool(name="perm", bufs=1) as perm,
            tc.tile_pool(name="wpool", bufs=1) as wpool,
        ):
            # Persistent SBUF tensors
            # qk_sb[:, j, 0:n] = Q.T area, [:, j, n:2n] = K.T area.
            # j=0: head0 on partitions 0:64, head1 on 64:128; j=1: head2 on 0:64.
            qk_sb = perm.tile([128, 2, 2 * n], F32R)
            # V (+ ones col per head) in [kv, d] layout: per kv-block of 128
            # tokens, 3 heads x (64 dims + ones col).
            v_sb = perm.tile([128, NKV, NH * (HD + 1)], F32R)

            wqkT_sb = wpool.tile([128, KE, QKDIM], F32R)
            wvT_sb = wpool.tile([128, KE, GDP], F32R)
            wpT_sb = wpool.tile([64, NH, E], F32R)
            bq_sb = wpool.tile([128, 2], F32)

            nc.sync.dma_start(wqkT_sb[:], wqkT.rearrange("(a p) c -> p a c", p=128))
            nc.sync.dma_start(wvT_sb[:], wvT.rearrange("(a p) c -> p a c", p=128))
            nc.sync.dma_start(wpT_sb[:], wpT[:])
            nc.sync.dma_start(bq_sb[:], bq.rearrange("a p -> p a"))

            # ones columns for the softmax-denominator trick
            ones_view = v_sb.rearrange("p a (h c) -> p a h c", c=HD + 1)[:, :, :, HD:]
            nc.vector.memset(ones_view.bitcast(F32), 1.0)

            # ---- One PSUM budget for everything (8 banks): tag "a" (2
            # banks) is time-shared by QKV-projection tiles and the output-
            # projection accumulators; "sc" 4 banks; "pv" 2 banks. This lets
            # the scheduler overlap the QKV projection with attention. ----
            with (
                tc.tile_pool(name="apsum", bufs=1, space="PSUM") as apsum,
                tc.tile_pool(name="bpsum", bufs=1, space="PSUM") as bpsum,
                tc.tile_pool(name="xpool", bufs=13) as xpool,
                tc.tile_pool(name="spool", bufs=3) as spool,
            ):
                for ng in range(NQG):
                    xts = []
                    for k in range(KE):
                        xt = xpool.tile([128, 512], F32R, tag="xt",
                                        name=f"xt{ng}_{k}")
                        nc.sync.dma_start(xt[:], xT[k * 128:(k + 1) * 128,
                                                    ng * 512:(ng + 1) * 512])
                        xts.append(xt)
                    qs = slice(ng * 512, (ng + 1) * 512)
                    ks = slice(n + ng * 512, n + (ng + 1) * 512)
                    for m in range(3):
                        psq = apsum.tile([128, 512], F32, tag="a", bufs=1,
                                         name=f"psq{ng}_{m}")
                        for k in range(KE):
                            nc.tensor.matmul(psq[:],
                                             wqkT_sb[:, k, m * 128:(m + 1) * 128],
                                             xts[k][:], start=(k == 0),
                                             stop=(k == KE - 1))
                        if m == 0:  # Q head0/1 + bias
                            nc.vector.tensor_scalar_add(qk_sb[:, 0, qs], psq[:],
                                                        bq_sb[:, 0:1])
                        elif m == 1:  # K head0/1
                            nc.vector.tensor_copy(qk_sb[:, 0, ks], psq[:])
                        else:  # m2 = [Q head2 ; K head2]
                            nc.vector.tensor_scalar_add(qk_sb[0:64, 1, qs],
                                                        psq[0:64, :],
                                                        bq_sb[0:64, 1:2])
                            # K head2 must live on partitions 0:64 (same as
                            # its Q). DMA can't read PSUM, so stage in SBUF
                            # then do a partition-shifting SBUF->SBUF DMA.
                            k2st = xpool.tile([128, 512], F32R, tag="k2st",
                                              bufs=2, name=f"k2st{ng}")
                            nc.vector.tensor_copy(k2st[64:128, :],
                                                  psq[64:128, :])
                            nc.sync.dma_start(qk_sb[0:64, 1, ks],
                                              k2st[64:128, :])
                    # V projection: 2 kv-blocks per 1-bank tile, j-outer
                    # so each bank hosts one accumulation group at a time
                    for vj in range(2):
                        psv = apsum.tile([128, 2, GDP], F32, tag="a", bufs=1,
                                         name=f"psv{ng}_{vj}")
                        for j in range(2):
                            jj = 2 * vj + j
                            for k in range(KE):
                                nc.tensor.matmul(
                                    psv[:, j, :],
                                    xts[k][:, jj * 128:(jj + 1) * 128],
                                    wvT_sb[:, k, :], start=(k == 0),
                                    stop=(k == KE - 1))
                        dst = v_sb[:, ng * 4 + 2 * vj:ng * 4 + 2 * vj + 2,
                                   :].rearrange(
                            "p a (h c) -> p a h c", c=HD + 1)[:, :, :, 0:HD]
                        src_ap = psv[:, :, 0:GD].rearrange(
                            "p a (h c) -> p a h c", c=HD)
                        nc.vector.tensor_copy(dst, src_ap)

                # ---- Stage B+C: software-pipelined attention ----
                # Single-head jobs (qg, h, kp), h0/h1 interleaved per kp so
                # consecutive scores matmuls hit disjoint PE row groups.
                # Scores are emitted at pipeline depth 2 (one full exp of
                # slack) so ACT never waits on PE.
                HEADS = {0: (0, 0), 1: (0, 64), 2: (1, 0)}  # h -> (jblk, pbase)
                jobs = []
                for qg in range(NQG):
                    for kp in range(NKV // 2):
                        jobs += [(qg, 0, kp), (qg, 1, kp), (qg, 2, kp)]
                pvp_tiles = {}
                yn = {}

                def emit_scores(qg, h, kp):
                    qsl = slice(qg * 512, (qg + 1) * 512)
                    jb, pb = HEADS[h]
                    sc = bpsum.tile([128, 2, 512], F32, tag="sc",
                                    bufs=2, name=f"sc{qg}_{h}_{kp}")
                    for j in range(2):
                        kv = 2 * kp + j
                        lhs = qk_sb[pb:pb + 64, jb,
                                    n + kv * 128:n + (kv + 1) * 128]
                        rhs = qk_sb[pb:pb + 64, jb, qsl]
                        nc.tensor.matmul(sc[:, j, :], lhs, rhs,
                                         start=True, stop=True)
                    return sc

                def emit_norm(qg, hh):
                    pvh = pvp_tiles[(qg, hh)]
                    r = spool.tile([1, 512], F32, tag="r",
                                   name=f"r{qg}_{hh}")
                    nc.vector.reciprocal(r[:], pvh[HD:HD + 1, :])
                    rb = spool.tile([64, 512], F32, tag="rb", bufs=2,
                                    name=f"rb{qg}_{hh}")
                    nc.gpsimd.partition_broadcast(rb[:], r[:])
                    yn[hh] = spool.tile([64, 512], F32R, tag="yn", bufs=6,
                                        name=f"yn{qg}_{hh}")
                    nc.vector.tensor_mul(yn[hh][:], pvh[0:HD, :], rb[:])

                def emit_proj(qg):
                    for f in range(2):
                        fw = 512 if f == 0 else E - 512
                        fsl = slice(f * 512, f * 512 + fw)
                        for qb in range(4):
                            pp = apsum.tile([128, fw], F32, tag="a", bufs=1,
                                            name=f"pp{qg}_{f}_{qb}")
                            for h in range(NH):
                                nc.tensor.matmul(
                                    pp[:], yn[h][:, qb * 128:(qb + 1) * 128],
                                    wpT_sb[:, h, fsl],
                                    start=(h == 0), stop=(h == NH - 1))
                            ost = spool.tile([128, fw], F32, tag="ost", bufs=4,
                                             name=f"ost{qg}_{f}_{qb}")
                            nc.vector.tensor_copy(ost[:], pp[:])
                            nc.sync.dma_start(
                                out[qg * 512 + qb * 128:
                                    qg * 512 + (qb + 1) * 128, fsl], ost[:])

                pending = [emit_scores(*jobs[0]), emit_scores(*jobs[1])]
                for idx, (qg, hh, kp) in enumerate(jobs):
                    if kp == 0:
                        if hh == 0:  # one 2-bank tensor for the h0/h1 pair
                            pvp = bpsum.tile([HD + 1, 2, 512], F32, tag="pv",
                                             bufs=1, name=f"pv{qg}_01")
                            pvp_tiles[(qg, 0)] = pvp[:, 0, :]
                            pvp_tiles[(qg, 1)] = pvp[:, 1, :]
                        elif hh == 2:
                            pv2 = bpsum.tile([HD + 1, 512], F32, tag="pv2",
                                             bufs=1, name=f"pv{qg}_2")
                            pvp_tiles[(qg, 2)] = pv2[:]
                    sc = pending.pop(0)
                    p = spool.tile([128, 2, 512], F32R, tag="p", bufs=6,
                                   name=f"p{qg}_{hh}_{kp}")
                    nc.scalar.activation(p[:], sc[:], AF.Exp, scale=0.125)
                    if idx + 2 < len(jobs):
                        pending.append(emit_scores(*jobs[idx + 2]))
                    for j in range(2):
                        kv = 2 * kp + j
                        nc.tensor.matmul(
                            pvp_tiles[(qg, hh)],
                            v_sb[:, kv, hh * (HD + 1):(hh + 1) * (HD + 1)],
                            p[:, j, :],
                            start=(kv == 0), stop=(kv == NKV - 1))
                    if kp == NKV // 2 - 1:
                        emit_norm(qg, hh)
                        if hh == 2:
                            emit_proj(qg)

    nc.finalize()
    return nc


def host_prep(x, w_qkv, b_qkv, w_proj, b_proj, n_tokens=N):
    """Build per-core input maps + the host-side combine closure."""
    x = np.asarray(x, np.float32)
    w_qkv = np.asarray(w_qkv, np.float32)
    b_qkv = np.asarray(b_qkv, np.float32)
    w_proj = np.asarray(w_proj, np.float32)
    b_proj = np.asarray(b_proj, np.float32)

    xT = [np.ascontiguousarray(x[b].T) for b in range(B)]  # [E, N]

    in_maps = []
    for c in range(8):
        b, g = divmod(c, M_GROUPS)
        base = g * NH * 3 * HD  # row offset of this group in w_qkv (576/group)
        # w_qkv row layout per head h: [h*192, +64)=Q, [+64, +128)=K, [+128, +192)=V
        wq = [w_qkv[base + i * 3 * HD: base + i * 3 * HD + HD] for i in range(NH)]
        wk = [w_qkv[base + i * 3 * HD + HD: base + i * 3 * HD + 2 * HD]
              for i in range(NH)]
        wv = [w_qkv[base + i * 3 * HD + 2 * HD: base + i * 3 * HD + 3 * HD]
              for i in range(NH)]
        bqv = [b_qkv[base + i * 3 * HD: base + i * 3 * HD + HD] for i in range(NH)]
        # m-tiles: m0=[Q0;Q1], m1=[K0;K1], m2=[Q2;K2]  (psum partition layout)
        wqkT = np.concatenate(
            [wq[0], wq[1], wk[0], wk[1], wq[2], wk[2]], axis=0).T  # [E, 384]
        wvT = np.concatenate(wv, axis=0).T  # [E, 192]
        wvT = np.concatenate([wvT, np.zeros((E, GDP - GD), np.float32)], axis=1)
        bq = np.zeros((2, 128), np.float32)
        bq[0, 0:HD] = bqv[0]
        bq[0, HD:2 * HD] = bqv[1]
        bq[1, 0:HD] = bqv[2]
        # wpT[d, h, f] = w_proj[f, g*192 + h*64 + d]
        wp = w_proj[:, g * GD:(g + 1) * GD]  # [768, 192]
        wpT = np.ascontiguousarray(
            wp.T.reshape(NH, HD, E).transpose(1, 0, 2))  # [64, 3, 768]
        in_maps.append({
            "xT": np.ascontiguousarray(xT[b]),
            "wqkT": np.ascontiguousarray(wqkT),
            "wvT": np.ascontiguousarray(wvT),
            "bq": bq,
            "wpT": wpT,
        })

    # fold V bias through the projection into the output bias
    bv_all = np.concatenate(
        [b_qkv[h * 3 * HD + 2 * HD: (h + 1) * 3 * HD] for h in range(H)])  # [768]
    b_eff = b_proj + w_proj @ bv_all

    def combine(results):
        out = np.empty((B, n_tokens, E), np.float32)
        for b in range(B):
            acc = results[b * M_GROUPS]["out"].astype(np.float32)
            for g in range(1, M_GROUPS):
                acc = acc + results[b * M_GROUPS + g]["out"]
            out[b] = acc + b_eff
        return out

    return in_maps, combine


_NC_CACHE = {}


def kernel(x, w_qkv, b_qkv, w_proj, b_proj):
    if "nc" not in _NC_CACHE:
        _NC_CACHE["nc"] = build_nc()
    nc = _NC_CACHE["nc"]
    in_maps, combine = host_prep(x, w_qkv, b_qkv, w_proj, b_proj)
    res = run_bass_kernel_spmd(nc, in_maps, core_ids=list(range(8)))
    return combine(res.results)


if __name__ == "__main__":
    rng = np.random.default_rng(0)
    inputs = {
        "x": rng.normal(size=(B, N, E)).astype(np.float32),
        "w_qkv": (rng.normal(size=(3 * E, E)) * 0.02).astype(np.float32),
        "b_qkv": (rng.normal(size=(3 * E,)) * 0.02).astype(np.float32),
        "w_proj": (rng.normal(size=(E, E)) * 0.02).astype(np.float32),
        "b_proj": (rng.normal(size=(E,)) * 0.02).astype(np.float32),
    }
    out = kernel(**inputs)
    print("out", out.shape, out.dtype, float(np.abs(out).mean()))
